# revision 30
# baseline (speedup 1.0000x reference)
"""CLUB loss kernel for Trainium2 (8 NeuronCores, SPMD row-sharded).

Math: the reference returns mean_i(pos_i - neg_i), a scalar.  Both the
pos and neg terms collapse into sums that never materialize the NxN
distance matrix:

  mean_pos = -0.5/N * (A - 2B + C)
      A = sum_{i,d} x[i,d]^2 * invv[i,d]
      B = sum_{i,d} x[i,d] * mu[i,d] * invv[i,d]
      C = sum_{i,d} mu[i,d]^2 * invv[i,d]
  mean_neg = -0.5 * (S_invv . S_x2 - 2 * S_muinvv . S_x + N*C) / N^2
      S_invv = sum_i invv[i,:]     S_muinvv = sum_i mu[i,:]*invv[i,:]
      S_x    = sum_j x[j,:]        S_x2     = sum_j x[j,:]^2
  loss = mean_pos - mean_neg

Each core handles 2048 rows (2 batches of x + matching mu/logvar rows)
and emits f32 partial sums; the host combines them in float64.

Layout: everything lives in the d-major layout (128, 1024): partition
q = (sub-slab b, dim d), free axis = row index within the sub-slab.
x arrives in this layout naturally (x[b] is (d, h*w) row-major); mu and
logvar are pre-transposed on the host as part of the shard layout.
With d on partitions every needed reduction is a free-axis row-sum, so
each quantity is one fused elementwise+accumulate instruction - no
on-chip transposes, no PSUM, no TensorEngine work at all (~20 compute
instructions per core).
"""

import sys

sys.path.insert(0, "/opt/trn_rl_repo")

import numpy as np
from contextlib import ExitStack

import concourse.bass as bass
import concourse.bacc as bacc
import concourse.tile as tile
from concourse import mybir
from concourse.bass_utils import run_bass_kernel_spmd

F32 = mybir.dt.float32
N_CORES = 8
B, D, H, W = 16, 64, 32, 32
HW = H * W                # 1024
N = B * HW                # 16384
NB = B // N_CORES         # 2 sub-slabs (batches) per core
ROWS = NB * HW            # 2048 rows per core
COLS = HW                 # free size of the (128, 1024) layout
# accum column map: quantity q, chunk c -> column q*NCH + c
QUANT = ["A", "B", "C", "Sx", "Sx2", "Sinvv", "Smuinvv"]
NCH = 2                   # accumulation chunks (bounds f32 chain length)
CW = COLS // NCH          # 512 columns per chunk


def build_nc() -> bass.Bass:
    nc = bacc.Bacc()
    xn = nc.dram_tensor("xn", [128, COLS], F32, kind="ExternalInput")
    mut = nc.dram_tensor("mut", [128, COLS], F32, kind="ExternalInput")
    lvt = nc.dram_tensor("lvt", [128, COLS], F32, kind="ExternalInput")
    accs = nc.dram_tensor("accs", [128, len(QUANT) * NCH], F32,
                          kind="ExternalOutput")

    with ExitStack() as ctx:
        tc = ctx.enter_context(tile.TileContext(nc))
        big = ctx.enter_context(tc.tile_pool(name="big", bufs=1))
        jp = ctx.enter_context(tc.tile_pool(name="jp", bufs=2))
        accp = ctx.enter_context(tc.tile_pool(name="accp", bufs=1))

        zerob = big.tile([128, 1], F32)
        nc.scalar.memzero(zerob[:])

        xb = big.tile([128, COLS], F32)
        mu = big.tile([128, COLS], F32)
        lv = big.tile([128, COLS], F32)
        # Split DMA issue across both HWDGE engines (SP + ACT) so the six
        # descriptor generations don't serialize, ordered by when compute
        # needs each chunk (lv gates the exp chain).
        sl0 = slice(0, CW)
        sl1 = slice(CW, COLS)
        nc.sync.dma_start(out=lv[:, sl0], in_=lvt[:, sl0])
        nc.scalar.dma_start(out=lv[:, sl1], in_=lvt[:, sl1])
        nc.sync.dma_start(out=xb[:, sl0], in_=xn[:, sl0])
        nc.scalar.dma_start(out=xb[:, sl1], in_=xn[:, sl1])
        nc.sync.dma_start(out=mu[:, sl0], in_=mut[:, sl0])
        nc.scalar.dma_start(out=mu[:, sl1], in_=mut[:, sl1])

        invv = big.tile([128, COLS], F32)
        muinvv = big.tile([128, COLS], F32)
        x2 = big.tile([128, COLS], F32)
        acc = accp.tile([128, len(QUANT) * NCH], F32)

        def col(q, c):
            return acc[:, QUANT.index(q) * NCH + c:QUANT.index(q) * NCH + c + 1]

        M = mybir.AluOpType.mult
        sls = [slice(h * CW, (h + 1) * CW) for h in range(NCH)]

        def act(q, h, out, in_, func, scale=1.0):
            nc.scalar.activation(
                out=out, in_=in_, func=func, bias=zerob[:], scale=scale,
                accum_out=col(q, h),
            )

        def stt(q, h, in0, in1):
            jd = jp.tile([128, CW], F32, tag="jd", name=f"jd_{q}{h}")
            nc.vector.scalar_tensor_tensor(
                out=jd[:], in0=in0[:, sls[h]], scalar=1.0, in1=in1[:, sls[h]],
                op0=M, op1=M, accum_out=col(q, h),
            )

        EXP = mybir.ActivationFunctionType.Exp
        SQ = mybir.ActivationFunctionType.Square

        # Emission order = engine program order; DMA-gated ops first on each
        # engine, cross-engine-gated ops (muinvv consumers) last.
        act("Sinvv", 0, invv[:, sls[0]], lv[:, sls[0]], EXP, scale=-1.0)
        act("Sx2", 0, x2[:, sls[0]], xb[:, sls[0]], SQ)
        jd = jp.tile([128, CW], F32, tag="jd", name="jd_sx0")
        nc.vector.tensor_scalar(
            out=jd[:], in0=xb[:, sls[0]], scalar1=1.0, scalar2=0.0,
            op0=M, op1=mybir.AluOpType.add, accum_out=col("Sx", 0),
        )
        nc.gpsimd.tensor_mul(muinvv[:, sls[0]], mu[:, sls[0]], invv[:, sls[0]])
        act("Sinvv", 1, invv[:, sls[1]], lv[:, sls[1]], EXP, scale=-1.0)
        act("Sx2", 1, x2[:, sls[1]], xb[:, sls[1]], SQ)
        stt("A", 0, x2, invv)
        jd = jp.tile([128, CW], F32, tag="jd", name="jd_sx1")
        nc.vector.tensor_scalar(
            out=jd[:], in0=xb[:, sls[1]], scalar1=1.0, scalar2=0.0,
            op0=M, op1=mybir.AluOpType.add, accum_out=col("Sx", 1),
        )
        nc.gpsimd.tensor_mul(muinvv[:, sls[1]], mu[:, sls[1]], invv[:, sls[1]])
        stt("B", 0, xb, muinvv)
        stt("C", 0, mu, muinvv)
        ja = jp.tile([128, CW], F32, tag="ja", name="ja_0")
        nc.scalar.activation(
            out=ja[:], in_=muinvv[:, sls[0]],
            func=mybir.ActivationFunctionType.Copy,
            bias=0.0, scale=1.0, accum_out=col("Smuinvv", 0),
        )
        stt("A", 1, x2, invv)
        stt("B", 1, xb, muinvv)
        stt("C", 1, mu, muinvv)
        ja = jp.tile([128, CW], F32, tag="ja", name="ja_1")
        nc.scalar.activation(
            out=ja[:], in_=muinvv[:, sls[1]],
            func=mybir.ActivationFunctionType.Copy,
            bias=0.0, scale=1.0, accum_out=col("Smuinvv", 1),
        )

        nc.sync.dma_start(out=accs[:, :], in_=acc[:])
    return nc


def _ensure_ntff_hook():
    """This image's antenv lacks axon_hooks; if tracing is requested
    (e.g. BASS_TRACE=1), run_bass_kernel_spmd would die on the import.
    Register the ctypes-based hook if available, else a None hook so
    tracing degrades gracefully."""
    import types

    if "antenv.axon_hooks" in sys.modules:
        return
    try:
        import antenv.axon_hooks  # noqa: F401
        return
    except ImportError:
        pass
    hook = None
    try:
        sys.path.insert(0, "/root/.axon_site")
        from trn_agent_boot.trn_boot import _ntff_profile_via_ctypes

        hook = _ntff_profile_via_ctypes("/opt/axon/libaxon_pjrt.so")
    except Exception:
        hook = None
    mod = types.ModuleType("antenv.axon_hooks")
    mod._hook = hook
    mod.get_axon_ntff_profile_hook = lambda: mod._hook
    mod.set_axon_ntff_profile_hook = lambda h: setattr(mod, "_hook", h)
    sys.modules["antenv.axon_hooks"] = mod


_ensure_ntff_hook()

_NC = None


def _get_nc():
    global _NC
    if _NC is None:
        _NC = build_nc()
        # bacc passes legalize multi-sync-wait instructions for TRN2 codegen
        _NC.compile()
    return _NC


def make_in_maps(x, mu, logvar):
    x = np.ascontiguousarray(np.asarray(x, dtype=np.float32))
    mu = np.asarray(mu, dtype=np.float32)
    lv = np.asarray(logvar, dtype=np.float32)
    in_maps = []
    for c in range(N_CORES):
        r0 = c * ROWS
        mu_t = np.concatenate(
            [mu[r0 + b * HW:r0 + (b + 1) * HW].T for b in range(NB)], axis=0
        )
        lv_t = np.concatenate(
            [lv[r0 + b * HW:r0 + (b + 1) * HW].T for b in range(NB)], axis=0
        )
        in_maps.append({
            "xn": x[c * NB:(c + 1) * NB].reshape(128, COLS),
            "mut": np.ascontiguousarray(mu_t),
            "lvt": np.ascontiguousarray(lv_t),
        })
    return in_maps


def combine(results) -> np.ndarray:
    nq = len(QUANT)
    tot = np.zeros((nq, 128), dtype=np.float64)
    for r in results:
        a = np.asarray(r["accs"], dtype=np.float64)  # (128, nq*NCH)
        for q in range(nq):
            tot[q] += a[:, q * NCH:(q + 1) * NCH].sum(axis=1)
    scal = {q: tot[i].sum() for i, q in enumerate(QUANT[:3])}
    vec = {q: tot[i].reshape(NB, D).sum(axis=0)
           for i, q in enumerate(QUANT) if i >= 3}
    A, Bs, C = scal["A"], scal["B"], scal["C"]
    mean_pos = -0.5 / N * (A - 2.0 * Bs + C)
    mean_D = (vec["Sinvv"] @ vec["Sx2"] - 2.0 * vec["Smuinvv"] @ vec["Sx"]
              + N * C) / float(N) ** 2
    loss = mean_pos + 0.5 * mean_D
    return np.array(loss, dtype=np.float32)


def kernel(x, mu, logvar, **_kwargs):
    nc = _get_nc()
    in_maps = make_in_maps(x, mu, logvar)
    res = run_bass_kernel_spmd(nc, in_maps, list(range(N_CORES)))
    return combine(res.results)


# revision 32
# speedup vs baseline: 1.0794x; 1.0794x over previous
"""CLUB loss kernel for Trainium2 (8 NeuronCores, SPMD row-sharded).

Math: the reference returns mean_i(pos_i - neg_i), a scalar.  Both the
pos and neg terms collapse into sums that never materialize the NxN
distance matrix:

  mean_pos = -0.5/N * (A - 2B + C)
      A = sum_{i,d} x[i,d]^2 * invv[i,d]
      B = sum_{i,d} x[i,d] * mu[i,d] * invv[i,d]
      C = sum_{i,d} mu[i,d]^2 * invv[i,d]
  mean_neg = -0.5 * (S_invv . S_x2 - 2 * S_muinvv . S_x + N*C) / N^2
      S_invv = sum_i invv[i,:]     S_muinvv = sum_i mu[i,:]*invv[i,:]
      S_x    = sum_j x[j,:]        S_x2     = sum_j x[j,:]^2
  loss = mean_pos - mean_neg

Each core handles 2048 rows (2 batches of x + matching mu/logvar rows)
and emits f32 partial sums; the host combines them in float64.

Layout: everything lives in the d-major layout (128, 1024): partition
q = (sub-slab b, dim d), free axis = row index within the sub-slab.
x arrives in this layout naturally (x[b] is (d, h*w) row-major); mu and
logvar are pre-transposed on the host as part of the shard layout.
With d on partitions every needed reduction is a free-axis row-sum, so
each quantity is one fused elementwise+accumulate instruction - no
on-chip transposes, no PSUM, no TensorEngine work at all (~20 compute
instructions per core).
"""

import sys

sys.path.insert(0, "/opt/trn_rl_repo")

import numpy as np
from contextlib import ExitStack

import concourse.bass as bass
import concourse.bacc as bacc
import concourse.tile as tile
from concourse import mybir
from concourse.bass_utils import run_bass_kernel_spmd

F32 = mybir.dt.float32
N_CORES = 8
B, D, H, W = 16, 64, 32, 32
HW = H * W                # 1024
N = B * HW                # 16384
NB = B // N_CORES         # 2 sub-slabs (batches) per core
ROWS = NB * HW            # 2048 rows per core
COLS = HW                 # free size of the (128, 1024) layout
# accum column map: quantity q, chunk c -> column q*NCH + c
QUANT = ["A", "B", "C", "Sx", "Sx2", "Sinvv", "Smuinvv"]
# Asymmetric chunks: small first chunk so compute starts as soon as the
# first bytes land, small last chunk so the dependency chain after the
# final DMA completes is short.
BOUNDS = [0, 256, 768, 1024]
NCH = len(BOUNDS) - 1


def build_nc() -> bass.Bass:
    nc = bacc.Bacc()
    xn = nc.dram_tensor("xn", [128, COLS], F32, kind="ExternalInput")
    mut = nc.dram_tensor("mut", [128, COLS], F32, kind="ExternalInput")
    lvt = nc.dram_tensor("lvt", [128, COLS], F32, kind="ExternalInput")
    accs = nc.dram_tensor("accs", [128, len(QUANT) * NCH], F32,
                          kind="ExternalOutput")

    with ExitStack() as ctx:
        tc = ctx.enter_context(tile.TileContext(nc))
        big = ctx.enter_context(tc.tile_pool(name="big", bufs=1))
        jp = ctx.enter_context(tc.tile_pool(name="jp", bufs=2))
        accp = ctx.enter_context(tc.tile_pool(name="accp", bufs=1))

        zerob = big.tile([128, 1], F32)
        nc.scalar.memzero(zerob[:])

        xb = big.tile([128, COLS], F32)
        mu = big.tile([128, COLS], F32)
        lv = big.tile([128, COLS], F32)
        sls = [slice(BOUNDS[h], BOUNDS[h + 1]) for h in range(NCH)]
        # Split DMA issue across both HWDGE engines (SP + ACT) so the
        # descriptor generations don't serialize; per chunk, lv first (it
        # heads the exp -> muinvv -> B/C chain), then mu, then x.
        qs = [nc.sync, nc.scalar]
        qi = 0
        for h in range(NCH):
            for t_dram, t_sbuf in ((lvt, lv), (mut, mu), (xn, xb)):
                qs[qi % 2].dma_start(
                    out=t_sbuf[:, sls[h]], in_=t_dram[:, sls[h]]
                )
                qi += 1

        invv = big.tile([128, COLS], F32)
        muinvv = big.tile([128, COLS], F32)
        x2 = big.tile([128, COLS], F32)
        acc = accp.tile([128, len(QUANT) * NCH], F32)

        def col(q, c):
            return acc[:, QUANT.index(q) * NCH + c:QUANT.index(q) * NCH + c + 1]

        M = mybir.AluOpType.mult

        def act(q, h, out, in_, func, scale=1.0):
            nc.scalar.activation(
                out=out, in_=in_, func=func, bias=zerob[:], scale=scale,
                accum_out=col(q, h),
            )

        def stt(q, h, in0, in1):
            jd = jp.tile([128, BOUNDS[h + 1] - BOUNDS[h]], F32, tag="jd",
                         name=f"jd_{q}{h}")
            nc.vector.scalar_tensor_tensor(
                out=jd[:], in0=in0[:, sls[h]], scalar=1.0, in1=in1[:, sls[h]],
                op0=M, op1=M, accum_out=col(q, h),
            )

        EXP = mybir.ActivationFunctionType.Exp
        SQ = mybir.ActivationFunctionType.Square

        # Emission order = engine program order.  Per chunk: the lv/mu
        # chain ops and the x-gated ops; the GPS-gated Smuinvv copies go
        # last so they never stall the ACT program.
        for h in range(NCH):
            act("Sinvv", h, invv[:, sls[h]], lv[:, sls[h]], EXP, scale=-1.0)
            nc.gpsimd.tensor_mul(
                muinvv[:, sls[h]], mu[:, sls[h]], invv[:, sls[h]]
            )
            act("Sx2", h, x2[:, sls[h]], xb[:, sls[h]], SQ)
            jd = jp.tile([128, BOUNDS[h + 1] - BOUNDS[h]], F32, tag="jd",
                         name=f"jd_sx{h}")
            nc.vector.tensor_scalar(
                out=jd[:], in0=xb[:, sls[h]], scalar1=1.0, scalar2=0.0,
                op0=M, op1=mybir.AluOpType.add, accum_out=col("Sx", h),
            )
            stt("A", h, x2, invv)
            stt("C", h, mu, muinvv)
            stt("B", h, xb, muinvv)
        for h in range(NCH):
            ja = jp.tile([128, BOUNDS[h + 1] - BOUNDS[h]], F32, tag="ja",
                         name=f"ja_{h}")
            nc.scalar.activation(
                out=ja[:], in_=muinvv[:, sls[h]],
                func=mybir.ActivationFunctionType.Copy,
                bias=0.0, scale=1.0, accum_out=col("Smuinvv", h),
            )

        nc.sync.dma_start(out=accs[:, :], in_=acc[:])
    return nc


def _ensure_ntff_hook():
    """This image's antenv lacks axon_hooks; if tracing is requested
    (e.g. BASS_TRACE=1), run_bass_kernel_spmd would die on the import.
    Register the ctypes-based hook if available, else a None hook so
    tracing degrades gracefully."""
    import types

    if "antenv.axon_hooks" in sys.modules:
        return
    try:
        import antenv.axon_hooks  # noqa: F401
        return
    except ImportError:
        pass
    hook = None
    try:
        sys.path.insert(0, "/root/.axon_site")
        from trn_agent_boot.trn_boot import _ntff_profile_via_ctypes

        hook = _ntff_profile_via_ctypes("/opt/axon/libaxon_pjrt.so")
    except Exception:
        hook = None
    mod = types.ModuleType("antenv.axon_hooks")
    mod._hook = hook
    mod.get_axon_ntff_profile_hook = lambda: mod._hook
    mod.set_axon_ntff_profile_hook = lambda h: setattr(mod, "_hook", h)
    sys.modules["antenv.axon_hooks"] = mod


_ensure_ntff_hook()

_NC = None


def _get_nc():
    global _NC
    if _NC is None:
        _NC = build_nc()
        # bacc passes legalize multi-sync-wait instructions for TRN2 codegen
        _NC.compile()
    return _NC


def make_in_maps(x, mu, logvar):
    x = np.ascontiguousarray(np.asarray(x, dtype=np.float32))
    mu = np.asarray(mu, dtype=np.float32)
    lv = np.asarray(logvar, dtype=np.float32)
    in_maps = []
    for c in range(N_CORES):
        r0 = c * ROWS
        mu_t = np.concatenate(
            [mu[r0 + b * HW:r0 + (b + 1) * HW].T for b in range(NB)], axis=0
        )
        lv_t = np.concatenate(
            [lv[r0 + b * HW:r0 + (b + 1) * HW].T for b in range(NB)], axis=0
        )
        in_maps.append({
            "xn": x[c * NB:(c + 1) * NB].reshape(128, COLS),
            "mut": np.ascontiguousarray(mu_t),
            "lvt": np.ascontiguousarray(lv_t),
        })
    return in_maps


def combine(results) -> np.ndarray:
    nq = len(QUANT)
    tot = np.zeros((nq, 128), dtype=np.float64)
    for r in results:
        a = np.asarray(r["accs"], dtype=np.float64)  # (128, nq*NCH)
        for q in range(nq):
            tot[q] += a[:, q * NCH:(q + 1) * NCH].sum(axis=1)
    scal = {q: tot[i].sum() for i, q in enumerate(QUANT[:3])}
    vec = {q: tot[i].reshape(NB, D).sum(axis=0)
           for i, q in enumerate(QUANT) if i >= 3}
    A, Bs, C = scal["A"], scal["B"], scal["C"]
    mean_pos = -0.5 / N * (A - 2.0 * Bs + C)
    mean_D = (vec["Sinvv"] @ vec["Sx2"] - 2.0 * vec["Smuinvv"] @ vec["Sx"]
              + N * C) / float(N) ** 2
    loss = mean_pos + 0.5 * mean_D
    return np.array(loss, dtype=np.float32)


def kernel(x, mu, logvar, **_kwargs):
    nc = _get_nc()
    in_maps = make_in_maps(x, mu, logvar)
    res = run_bass_kernel_spmd(nc, in_maps, list(range(N_CORES)))
    return combine(res.results)


# revision 35
# speedup vs baseline: 1.1158x; 1.0337x over previous
"""CLUB loss kernel for Trainium2 (8 NeuronCores, SPMD row-sharded).

Math: the reference returns mean_i(pos_i - neg_i), a scalar.  Both the
pos and neg terms collapse into sums that never materialize the NxN
distance matrix:

  mean_pos = -0.5/N * (A - 2B + C)
      A = sum_{i,d} x[i,d]^2 * invv[i,d]
      B = sum_{i,d} x[i,d] * mu[i,d] * invv[i,d]
      C = sum_{i,d} mu[i,d]^2 * invv[i,d]
  mean_neg = -0.5 * (S_invv . S_x2 - 2 * S_muinvv . S_x + N*C) / N^2
      S_invv = sum_i invv[i,:]     S_muinvv = sum_i mu[i,:]*invv[i,:]
      S_x    = sum_j x[j,:]        S_x2     = sum_j x[j,:]^2
  loss = mean_pos - mean_neg

Each core handles 2048 rows (2 batches of x + matching mu/logvar rows)
and emits f32 partial sums; the host combines them in float64.

Layout: everything lives in the d-major layout (128, 1024): partition
q = (sub-slab b, dim d), free axis = row index within the sub-slab.
x arrives in this layout naturally (x[b] is (d, h*w) row-major); mu and
logvar are pre-transposed on the host as part of the shard layout.
With d on partitions every needed reduction is a free-axis row-sum, so
each quantity is one fused elementwise+accumulate instruction - no
on-chip transposes, no PSUM, no TensorEngine work at all (~20 compute
instructions per core).
"""

import sys

sys.path.insert(0, "/opt/trn_rl_repo")

import numpy as np
from contextlib import ExitStack

import concourse.bass as bass
import concourse.bacc as bacc
import concourse.tile as tile
from concourse import mybir
from concourse.bass_utils import run_bass_kernel_spmd

F32 = mybir.dt.float32
N_CORES = 8
B, D, H, W = 16, 64, 32, 32
HW = H * W                # 1024
N = B * HW                # 16384
NB = B // N_CORES         # 2 sub-slabs (batches) per core
ROWS = NB * HW            # 2048 rows per core
COLS = HW                 # free size of the (128, 1024) layout
# accum column map: quantity q, chunk c -> column q*NCH + c
QUANT = ["A", "B", "C", "Sx", "Sx2", "Sinvv", "Smuinvv"]
# Asymmetric chunks: small first chunk so compute starts as soon as the
# first bytes land, small last chunk so the dependency chain after the
# final DMA completes is short.
BOUNDS = [0, 256, 768, 1024]
NCH = len(BOUNDS) - 1


def build_nc() -> bass.Bass:
    nc = bacc.Bacc()
    # one fully-contiguous DRAM tensor per (input, chunk) so every load is
    # a pure 1D burst (the host packs chunks during shard prep)
    xn, mut, lvt = ({
        h: nc.dram_tensor(f"{nm}{h}", [128, BOUNDS[h + 1] - BOUNDS[h]], F32,
                          kind="ExternalInput")
        for h in range(NCH)
    } for nm in ("xn", "mut", "lvt"))
    accs = nc.dram_tensor("accs", [128, len(QUANT) * NCH], F32,
                          kind="ExternalOutput")

    with ExitStack() as ctx:
        tc = ctx.enter_context(tile.TileContext(nc))
        big = ctx.enter_context(tc.tile_pool(name="big", bufs=1))
        jp = ctx.enter_context(tc.tile_pool(name="jp", bufs=2))
        accp = ctx.enter_context(tc.tile_pool(name="accp", bufs=1))

        zerob = big.tile([128, 1], F32)
        nc.scalar.memzero(zerob[:])

        xb = big.tile([128, COLS], F32)
        mu = big.tile([128, COLS], F32)
        lv = big.tile([128, COLS], F32)
        sls = [slice(BOUNDS[h], BOUNDS[h + 1]) for h in range(NCH)]
        # Split DMA issue across both HWDGE engines (SP + ACT) so the
        # descriptor generations don't serialize; per chunk, lv first (it
        # heads the exp -> muinvv -> B/C chain), then mu, then x.
        qs = [nc.sync, nc.scalar]
        qi = 0
        for h in range(NCH):
            for t_dram, t_sbuf in ((lvt, lv), (mut, mu), (xn, xb)):
                qs[qi % 2].dma_start(
                    out=t_sbuf[:, sls[h]], in_=t_dram[h][:, :]
                )
                qi += 1

        invv = big.tile([128, COLS], F32)
        muinvv = big.tile([128, COLS], F32)
        x2 = big.tile([128, COLS], F32)
        acc = accp.tile([128, len(QUANT) * NCH], F32)

        def col(q, c):
            return acc[:, QUANT.index(q) * NCH + c:QUANT.index(q) * NCH + c + 1]

        M = mybir.AluOpType.mult

        def act(q, h, out, in_, func, scale=1.0):
            nc.scalar.activation(
                out=out, in_=in_, func=func, bias=zerob[:], scale=scale,
                accum_out=col(q, h),
            )

        def stt(q, h, in0, in1):
            jd = jp.tile([128, BOUNDS[h + 1] - BOUNDS[h]], F32, tag="jd",
                         name=f"jd_{q}{h}")
            nc.vector.scalar_tensor_tensor(
                out=jd[:], in0=in0[:, sls[h]], scalar=1.0, in1=in1[:, sls[h]],
                op0=M, op1=M, accum_out=col(q, h),
            )

        EXP = mybir.ActivationFunctionType.Exp
        SQ = mybir.ActivationFunctionType.Square

        # Emission order = engine program order.  Per chunk: the lv/mu
        # chain ops and the x-gated ops; the GPS-gated Smuinvv copies go
        # last so they never stall the ACT program.
        for h in range(NCH):
            act("Sinvv", h, invv[:, sls[h]], lv[:, sls[h]], EXP, scale=-1.0)
            nc.gpsimd.tensor_mul(
                muinvv[:, sls[h]], mu[:, sls[h]], invv[:, sls[h]]
            )
            act("Sx2", h, x2[:, sls[h]], xb[:, sls[h]], SQ)
            jd = jp.tile([128, BOUNDS[h + 1] - BOUNDS[h]], F32, tag="jd",
                         name=f"jd_sx{h}")
            nc.vector.tensor_scalar(
                out=jd[:], in0=xb[:, sls[h]], scalar1=1.0, scalar2=0.0,
                op0=M, op1=mybir.AluOpType.add, accum_out=col("Sx", h),
            )
            stt("A", h, x2, invv)
            stt("C", h, mu, muinvv)
            stt("B", h, xb, muinvv)
        for h in range(NCH):
            ja = jp.tile([128, BOUNDS[h + 1] - BOUNDS[h]], F32, tag="ja",
                         name=f"ja_{h}")
            nc.scalar.activation(
                out=ja[:], in_=muinvv[:, sls[h]],
                func=mybir.ActivationFunctionType.Copy,
                bias=0.0, scale=1.0, accum_out=col("Smuinvv", h),
            )

        nc.sync.dma_start(out=accs[:, :], in_=acc[:])
    return nc


def _ensure_ntff_hook():
    """This image's antenv lacks axon_hooks; if tracing is requested
    (e.g. BASS_TRACE=1), run_bass_kernel_spmd would die on the import.
    Register the ctypes-based hook if available, else a None hook so
    tracing degrades gracefully."""
    import types

    if "antenv.axon_hooks" in sys.modules:
        return
    try:
        import antenv.axon_hooks  # noqa: F401
        return
    except ImportError:
        pass
    hook = None
    try:
        sys.path.insert(0, "/root/.axon_site")
        from trn_agent_boot.trn_boot import _ntff_profile_via_ctypes

        hook = _ntff_profile_via_ctypes("/opt/axon/libaxon_pjrt.so")
    except Exception:
        hook = None
    mod = types.ModuleType("antenv.axon_hooks")
    mod._hook = hook
    mod.get_axon_ntff_profile_hook = lambda: mod._hook
    mod.set_axon_ntff_profile_hook = lambda h: setattr(mod, "_hook", h)
    sys.modules["antenv.axon_hooks"] = mod


_ensure_ntff_hook()

_NC = None


def _get_nc():
    global _NC
    if _NC is None:
        _NC = build_nc()
        # bacc passes legalize multi-sync-wait instructions for TRN2 codegen
        _NC.compile()
    return _NC


def make_in_maps(x, mu, logvar):
    x = np.ascontiguousarray(np.asarray(x, dtype=np.float32))
    mu = np.asarray(mu, dtype=np.float32)
    lv = np.asarray(logvar, dtype=np.float32)
    in_maps = []
    for c in range(N_CORES):
        r0 = c * ROWS
        mu_t = np.concatenate(
            [mu[r0 + b * HW:r0 + (b + 1) * HW].T for b in range(NB)], axis=0
        )
        lv_t = np.concatenate(
            [lv[r0 + b * HW:r0 + (b + 1) * HW].T for b in range(NB)], axis=0
        )
        x_t = x[c * NB:(c + 1) * NB].reshape(128, COLS)
        m = {}
        for h in range(len(BOUNDS) - 1):
            sl = slice(BOUNDS[h], BOUNDS[h + 1])
            m[f"xn{h}"] = np.ascontiguousarray(x_t[:, sl])
            m[f"mut{h}"] = np.ascontiguousarray(mu_t[:, sl])
            m[f"lvt{h}"] = np.ascontiguousarray(lv_t[:, sl])
        in_maps.append(m)
    return in_maps


def combine(results) -> np.ndarray:
    nq = len(QUANT)
    tot = np.zeros((nq, 128), dtype=np.float64)
    for r in results:
        a = np.asarray(r["accs"], dtype=np.float64)  # (128, nq*NCH)
        for q in range(nq):
            tot[q] += a[:, q * NCH:(q + 1) * NCH].sum(axis=1)
    scal = {q: tot[i].sum() for i, q in enumerate(QUANT[:3])}
    vec = {q: tot[i].reshape(NB, D).sum(axis=0)
           for i, q in enumerate(QUANT) if i >= 3}
    A, Bs, C = scal["A"], scal["B"], scal["C"]
    mean_pos = -0.5 / N * (A - 2.0 * Bs + C)
    mean_D = (vec["Sinvv"] @ vec["Sx2"] - 2.0 * vec["Smuinvv"] @ vec["Sx"]
              + N * C) / float(N) ** 2
    loss = mean_pos + 0.5 * mean_D
    return np.array(loss, dtype=np.float32)


def kernel(x, mu, logvar, **_kwargs):
    nc = _get_nc()
    in_maps = make_in_maps(x, mu, logvar)
    res = run_bass_kernel_spmd(nc, in_maps, list(range(N_CORES)))
    return combine(res.results)
